# revision 1
# baseline (speedup 1.0000x reference)
"""KoLeo loss kernel for 8 Trainium2 NeuronCores — symmetric (half-matrix)
variant.

Reference computation (B=16384, D=1024):
    xn  = x / max(||x||_2, 1e-12)          # row L2-normalize
    sim = xn @ xn.T                        # B x B cosine similarity
    max_sim[i] = max_{j != i} sim[i, j]    # nearest neighbor (excl. self)
    out = -mean(log(sqrt(2 - 2*max_sim + 1e-8)))

sim is symmetric, so only the upper triangle of 512x512 super-blocks is
computed (~53% of the matmul work of the dense baseline). Each computed
super-block (I, J), I < J, serves rows of I via its row-max and rows of J
via its column-max. Per-super-block epilogue (every stage fits under the
Tensor engine's ~3.6 us of matmul work; Pool cannot touch PSUM on TRN2
and its partition_all_reduce is too slow at ~3.4 ns/output element, so
Pool stays idle — fewer busy engines also eases the chip power throttle):
  - ACT: sole bulk PSUM consumer — one [128, 2048] copy stages the whole
    4-bank tile to fp16 SBUF (~2.5 us: 1 elem/cycle + ~0.7 us fixed), so
    the PE recycles its two psum buffers quickly.
  - DVE: row-max reduce of the staged tile (reduce has no 16-bit fast
    mode, ~2.2 us), then two fp16 2x tensor_max folds to [128, 512].
  - DMA: ships the folded tile to DRAM (~128KB/block).
  - Host: 128-partition max + scatter into the global row maxima.

Work distribution ("pencil window", SPMD-uniform): global super-row G is
owned by core c = G % 8. Each core holds its 4 owned super-rows resident
(local positions 8a after a per-core rotation of x.T columns by 512*c) and
computes super-blocks (I, I+w mod 32) for w = 0..16 (a in {0,1}) or
w = 0..15 (a in {2,3}). Every unordered pair of super-blocks is covered
exactly once across the fleet (528 total); w=0 is the diagonal block,
where a -2*scale^2*I constant is added at the self-similarity positions
before the row max so the self-match never wins. Per core: 66
super-blocks = 1056 fp8 DoubleRow matmuls (~229 us at the 157 TF/s fp8
roofline, vs 437 us for the dense baseline).

The input is laid out host-side as [J, p, k, j] (so each 512-column block
is one contiguous [128 partitions x 4KB] slab) and each rhs block loads
with a single DMA — ~130 DMA issues total instead of ~530.

Host: pre-normalizes rows (f64), scales by 8 and casts to fp8e4m3,
pre-rotates/retiles per core; post-merges row/column maxima across cores
and applies the scalar log epilogue in f64.
"""

import sys

if "/opt/trn_rl_repo" not in sys.path:
    sys.path.insert(0, "/opt/trn_rl_repo")

import numpy as np
import ml_dtypes

import concourse.bass as bass  # noqa: F401  (import keeps bass registered)
import concourse.mybir as mybir
import concourse.tile as tile
from concourse import bacc
from concourse.bass_utils import run_bass_kernel_spmd

P = 128          # SBUF partitions
NBLK = 512       # super-block side (= one PSUM bank of f32 per 128 rows)
EPS = 1e-8

B = 16384        # rows of x
D = 1024         # feature dim
N_CORES = 8
NSB = B // NBLK  # 32 super-blocks per matrix side
KCH = D // P     # 8 contraction chunks of 128
KSTEP = 2        # fp8 DoubleRow: K chunks of 256 per matmul
NA = 4           # owned super-rows per core (global stride 8)
FP8_SCALE = 8.0


def _windows():
    """Program-order (a, w) list. w=0 is the diagonal super-block.

    a in {0,1} get w up to 16, a in {2,3} up to 15: the distance-16 pairs
    {i, i+16} are covered once by the a0/a1 windows (i = c + 8*a0), so the
    a2/a3 windows stop at 15. Total 66 super-blocks per core; the union of
    (owned I, I+w) over all cores covers every unordered block pair once.
    """
    out = []
    for a in range(NA):
        wmax = 16 if a < 2 else 15
        for w in range(wmax + 1):
            out.append((a, w))
    return out


def _upper_order():
    """Program-order list of the 62 strictly-upper (a, w) super-blocks."""
    return [(a, w) for a, w in _windows() if w > 0]


N_UP = len(_upper_order())  # 62


def build_nc():
    """Build the per-core SPMD Bass program.

    Inputs :  xt     [NSB*P, KCH*NBLK] fp8e4m3 — normalized, scaled x.T,
              retiled as [J, p, k, j] and column-rotated by 512*c so owned
              super-rows sit at local block 8a.
              negeye [P, P] f32 — the constant -2*scale^2 * I
    Outputs:  rowmax [P, 16]         f32 — [p, 4a+r] = row-max over the
              computed window for local row 4096a + 128r + p
              colmax [N_UP*P, NBLK]  f16 — per strictly-upper super-block
              (program order), the 4-chunk-folded [128, 512] tile; the
              host reduces the 128 partitions to the block column max.
    """
    f32 = mybir.dt.float32
    f16 = mybir.dt.float16
    fp8 = mybir.dt.float8e4
    ngrp = KCH // KSTEP

    nc = bacc.Bacc("TRN2", target_bir_lowering=False, debug=False,
                   num_devices=N_CORES)
    xt = nc.dram_tensor("xt", [NSB * P, KCH * NBLK], fp8,
                        kind="ExternalInput")
    negeye = nc.dram_tensor("negeye", [P, P], f32, kind="ExternalInput")
    rowmax_d = nc.dram_tensor("rowmax", [P, NA * 4], f32,
                              kind="ExternalOutput")
    colmax_d = nc.dram_tensor("colmax", [N_UP * P, NBLK], f16,
                              kind="ExternalOutput")
    xt_ap = xt[:]
    colmax_ap = colmax_d[:]

    with tile.TileContext(nc) as tc:
        with (
            tc.tile_pool(name="lhs", bufs=1) as lhs_pool,
            tc.tile_pool(name="rhs", bufs=3) as rhs_pool,
            tc.tile_pool(name="psum", bufs=2, space="PSUM") as psum_pool,
            tc.tile_pool(name="stage", bufs=4) as stage_pool,
            tc.tile_pool(name="stats", bufs=1) as stats_pool,
        ):
            dma_eng = [nc.sync, nc.scalar]
            ndma = 0

            lhs_tiles = []
            for a in range(NA):
                t = lhs_pool.tile([P, KCH, NBLK], fp8, name=f"lhs{a}",
                                  tag=f"lhs{a}")
                lhs_tiles.append(t)
                rows = slice(8 * a * P, (8 * a + 1) * P)
                dma_eng[ndma % 2].dma_start(t[:], xt_ap[rows, :])
                ndma += 1
            eye = stats_pool.tile([P, P], f32, name="eye")
            nc.sync.dma_start(eye[:], negeye[:])
            maxt = [
                stats_pool.tile([P, 4, 17], f32, name=f"maxt{a}",
                                tag=f"maxt{a}")
                for a in range(NA)
            ]
            rowmax_sb = stats_pool.tile([P, NA * 4], f32, name="rowmax_sb")

            sb_idx = 0  # strictly-upper super-block output row
            for a, w in _windows():
                J = (8 * a + w) % NSB
                if w == 0:
                    rt = None  # rhs block is the resident lhs tile itself
                else:
                    rt = rhs_pool.tile([P, KCH, NBLK], fp8, name="rt",
                                       tag="rt")
                    dma_eng[ndma % 2].dma_start(
                        rt[:], xt_ap[J * P:(J + 1) * P, :])
                    ndma += 1

                # One 4-bank psum tile per super-block (chunks r = 0..3).
                ps = psum_pool.tile([P, 4, NBLK], f32, name="ps", tag="ps")
                for r in range(4):
                    for g in range(ngrp):
                        ks = slice(KSTEP * g, KSTEP * (g + 1))
                        rhs = (lhs_tiles[a][:, ks, :] if rt is None
                               else rt[:, ks, :])
                        nc.tensor.matmul(
                            ps[:, r, :],
                            lhs_tiles[a][:, ks, r * P:(r + 1) * P],
                            rhs,
                            start=(g == 0),
                            stop=(g == ngrp - 1),
                            perf_mode=mybir.MatmulPerfMode.DoubleRow,
                        )

                if w == 0:
                    # self-similarity of chunk r lives at [p, r*P + p]:
                    # add -2*scale^2*I so the self-match never wins.
                    for r in range(4):
                        sl = ps[:, r, r * P:(r + 1) * P]
                        nc.vector.tensor_add(out=sl, in0=sl, in1=eye[:])

                # ACT is the sole bulk PSUM consumer: one [128, 2048] copy
                # stages the whole tile to fp16 SBUF (~2.5 us), so the PE
                # can recycle the psum buffer quickly. DVE reduces the
                # staged copy instead of PSUM.
                st = stage_pool.tile([P, 4, NBLK], f16, name="st", tag="st")
                nc.scalar.copy(st[:], ps[:])

                nc.vector.reduce_max(
                    out=maxt[a][:, :, w:w + 1],
                    in_=st[:],
                    axis=mybir.AxisListType.X,
                    op=mybir.AluOpType.max,
                )

                if w > 0:
                    stm = stage_pool.tile([P, 2, NBLK], f16, name="stm",
                                          tag="stm", bufs=6)
                    nc.vector.tensor_max(out=stm[:], in0=st[:, 0:2, :],
                                         in1=st[:, 2:4, :])
                    mg = stage_pool.tile([P, NBLK], f16, name="mg",
                                         tag="mg", bufs=8)
                    nc.vector.tensor_max(out=mg[:], in0=stm[:, 0, :],
                                         in1=stm[:, 1, :])
                    # Ship the folded [128, 512] tile; the host does the
                    # 128-partition max — cheaper than Pool's
                    # partition_all_reduce (~3.9 us/block on the Q7s).
                    dma_eng[ndma % 2].dma_start(
                        colmax_ap[sb_idx * P:(sb_idx + 1) * P, :], mg[:])
                    ndma += 1
                    sb_idx += 1

            for a in range(NA):
                nw = 17 if a < 2 else 16
                nc.vector.reduce_max(
                    out=rowmax_sb[:, 4 * a:4 * a + 4],
                    in_=maxt[a][:, :, 0:nw],
                    axis=mybir.AxisListType.X,
                    op=mybir.AluOpType.max,
                )
            nc.sync.dma_start(rowmax_d[:], rowmax_sb[:])

    nc.compile()
    return nc


def prepare_inputs(x):
    """Host prep: normalize (f64), scale+cast fp8, retile, rotate."""
    xd = np.asarray(x, dtype=np.float64)
    norms = np.sqrt(np.einsum("ij,ij->i", xd, xd))
    np.maximum(norms, 1e-12, out=norms)
    xn = xd / norms[:, None]
    xnt = (xn.T * FP8_SCALE).astype(ml_dtypes.float8_e4m3)  # [D, B]
    # retile to [J, p, k, j]: xt_r[J, p, k, j] = xnt[k*128+p, J*512+j]
    xt_r = np.ascontiguousarray(
        xnt.reshape(KCH, P, NSB, NBLK).transpose(2, 1, 0, 3))
    negeye = np.ascontiguousarray(
        (-2.0 * FP8_SCALE * FP8_SCALE) * np.eye(P, dtype=np.float32))
    in_maps = []
    for c in range(N_CORES):
        rot = (np.concatenate([xt_r[c:], xt_r[:c]], axis=0) if c
               else xt_r)
        in_maps.append({
            "xt": np.ascontiguousarray(rot).reshape(NSB * P, KCH * NBLK),
            "negeye": negeye,
        })
    return in_maps


def postprocess(results):
    """Merge per-core row/column maxima and apply the scalar epilogue."""
    inv = 1.0 / (FP8_SCALE * FP8_SCALE)
    order = _upper_order()
    maxsim = np.full(B, -np.inf, dtype=np.float64)
    for c in range(N_CORES):
        rm = np.asarray(results[c]["rowmax"], dtype=np.float64)  # [P, 16]
        for a in range(NA):
            for r in range(4):
                g0 = (c + 8 * a) * NBLK + r * P  # global row of partition 0
                sl = slice(g0, g0 + P)
                np.maximum(maxsim[sl], rm[:, 4 * a + r], out=maxsim[sl])
        cmx = np.asarray(results[c]["colmax"]).astype(np.float32)
        # fold the 128 partitions on the host
        cmx = cmx.reshape(N_UP, P, NBLK).max(axis=1).astype(np.float64)
        for s, (a, w) in enumerate(order):
            g0 = ((8 * a + w + c) % NSB) * NBLK
            sl = slice(g0, g0 + NBLK)
            np.maximum(maxsim[sl], cmx[s], out=maxsim[sl])
    d2 = 2.0 - 2.0 * (maxsim * inv) + EPS
    loss = -0.5 * np.mean(np.log(d2))
    return np.array(loss, dtype=np.float32)


_NC_CACHE = {}


def _get_nc():
    if "nc" not in _NC_CACHE:
        _NC_CACHE["nc"] = build_nc()
    return _NC_CACHE["nc"]


def kernel(x, **_ignored):
    import time

    nc = _get_nc()
    in_maps = prepare_inputs(x)
    last_exc = None
    for attempt in range(3):
        try:
            res = run_bass_kernel_spmd(nc, in_maps,
                                       core_ids=list(range(N_CORES)))
            return postprocess(res.results)
        except Exception as exc:  # transient NRT/tunnel hiccups
            last_exc = exc
            if attempt < 2:
                time.sleep(30)  # a wedged exec unit takes a while to heal
    raise last_exc


if __name__ == "__main__":
    x = np.random.default_rng(0).standard_normal((B, D), dtype=np.float32)
    print(kernel(x))



# revision 12
# speedup vs baseline: 1.2163x; 1.2163x over previous
"""KoLeo loss kernel for 8 Trainium2 NeuronCores — symmetric (half-matrix)
variant, v2.

Reference computation (B=16384, D=1024):
    xn  = x / max(||x||_2, 1e-12)          # row L2-normalize
    sim = xn @ xn.T                        # B x B cosine similarity
    max_sim[i] = max_{j != i} sim[i, j]    # nearest neighbor (excl. self)
    out = -mean(log(sqrt(2 - 2*max_sim + 1e-8)))

sim is symmetric, so only the upper triangle of 512x512 super-blocks is
computed. Work distribution ("pencil window", SPMD-uniform): global
super-row G is owned by core c = G % 8. Each core holds its 4 owned
super-rows resident and computes super-blocks (I, I+w mod 32) for
w = 0..16 (a in {0,1}) or w = 0..15 (a in {2,3}); every unordered pair
of super-blocks is covered exactly once across the fleet (528 total).

v2 changes vs v1 (which ran PE and DVE both ~90% busy at ~3.85us/block,
258-308us total depending on the chip power state):
  - Diagonal (w=0) super-blocks only compute their upper-triangle
    128-row chunks (rows chunk r x cols 128r..512): 5120 instead of
    8192 PE cycles per diag block. A ragged column-max tile (cols
    128..512, max over the computed rows, self-sims masked to -130+64)
    is shipped so the skipped lower-triangle pairs stay covered.
  - DVE row-max restructure: instead of a full 2.2us tensor_reduce per
    block, a running row-max accumulator racc[a] takes one f16 2x-mode
    tensor_max per block (~1.1us); the expensive reduce happens once
    per window. DVE drops from ~3.85us/block (co-bottleneck with PE)
    to ~2.1us/block. (tensor_tensor_reduce would fuse this further but
    faults the exec unit on this runtime — verified by HW probe.)
  - rhs blocks at local index 8k (w = 8, 16) are the resident lhs
    tiles — 6 fewer 512KB HBM reads.
  - All DMA issues move off the Scalar queue (ACT was doing 44us of
    DMA_DIRECT2D issue on top of its 156us of PSUM-drain copies):
    inputs on Sync, outputs on GpSimd (otherwise idle).
  - ~24 tiny f16 warm-up matmuls on the eye tile run during the input
    DMA window so the PE's HAM clock-gate (cold 1.2 GHz for the first
    ~3.4us of activity) warms up on garbage instead of real work.

Per-block engine budget (warm, 2.4 GHz PE): PE 3.41us (16 fp8 DoubleRow
matmuls, N=512, zero inter-MM bubble measured), ACT 2.36us (PSUM->f16
SBUF stage copy, sole bulk PSUM consumer), DVE ~2.1us, leaving PE the
sole bottleneck at ~220us/core + start/tail.

Host: pre-normalizes rows (f64), scales by 8 and casts to fp8e4m3,
pre-rotates/retiles per core; post-merges row/column maxima across
cores and applies the scalar log epilogue in f64.
"""

import sys

if "/opt/trn_rl_repo" not in sys.path:
    sys.path.insert(0, "/opt/trn_rl_repo")

import numpy as np
import ml_dtypes

import concourse.bass as bass  # noqa: F401  (import keeps bass registered)
import concourse.mybir as mybir
import concourse.tile as tile
from concourse import bacc
from concourse.bass_utils import run_bass_kernel_spmd

P = 128          # SBUF partitions
NBLK = 512       # super-block side (= one PSUM bank of f32 per 128 rows)
EPS = 1e-8

B = 16384        # rows of x
D = 1024         # feature dim
N_CORES = 8
NSB = B // NBLK  # 32 super-blocks per matrix side
KCH = D // P     # 8 contraction chunks of 128
KSTEP = 2        # fp8 DoubleRow: K chunks of 256 per matmul
NA = 4           # owned super-rows per core (global stride 8)
FP8_SCALE = 8.0
EYE_VAL = -130.0  # added at self-sim positions (value 64) before maxes
N_WARMUP = 24    # f16 eye matmuls to heat the PE HAM clock-gate


def _windows():
    """Program-order (a, w) list. w=0 is the diagonal super-block.

    a in {0,1} get w up to 16, a in {2,3} up to 15: the distance-16 pairs
    {i, i+16} are covered once by the a0/a1 windows (i = c + 8*a0), so the
    a2/a3 windows stop at 15. Total 66 super-blocks per core; the union of
    (owned I, I+w) over all cores covers every unordered block pair once.
    """
    out = []
    for a in range(NA):
        wmax = 16 if a < 2 else 15
        for w in range(wmax + 1):
            out.append((a, w))
    return out


N_SLOTS = len(_windows())  # 66: every block ships a colmax tile now
DIAG_W = 384               # diag colmax covers block cols 128..512


def build_nc():
    """Build the per-core SPMD Bass program.

    Inputs :  xt     [NSB*P, KCH*NBLK] fp8e4m3 — normalized, scaled x.T,
              retiled as [J, p, k, j] and column-rotated by 512*c so owned
              super-rows sit at local block 8a.
              eyef16 [P, P] f16 — the constant EYE_VAL * I
    Outputs:  rowmax [P, 16]          f32 — [p, 4a+r] = row-max over the
              computed window for local row 4096a + 128r + p
              colmax [N_SLOTS*P, NBLK] f16 — per super-block (program
              order), the r-chunk-folded [128, 512] column-max tile (for
              diagonal blocks only cols 0:384 are valid, covering block
              cols 128..512); the host reduces the 128 partitions.
    """
    f32 = mybir.dt.float32
    f16 = mybir.dt.float16
    fp8 = mybir.dt.float8e4
    ngrp = KCH // KSTEP

    nc = bacc.Bacc("TRN2", target_bir_lowering=False, debug=False,
                   num_devices=N_CORES)
    xt = nc.dram_tensor("xt", [NSB * P, KCH * NBLK], fp8,
                        kind="ExternalInput")
    eyed = nc.dram_tensor("eyef16", [P, P], f16, kind="ExternalInput")
    rowmax_d = nc.dram_tensor("rowmax", [P, NA * 4], f32,
                              kind="ExternalOutput")
    colmax_d = nc.dram_tensor("colmax", [N_SLOTS * P, NBLK], f16,
                              kind="ExternalOutput")
    xt_ap = xt[:]
    colmax_ap = colmax_d[:]

    with tile.TileContext(nc) as tc:
        with (
            tc.tile_pool(name="lhs", bufs=1) as lhs_pool,
            tc.tile_pool(name="rhs", bufs=3) as rhs_pool,
            tc.tile_pool(name="psum", bufs=2, space="PSUM") as psum_pool,
            tc.tile_pool(name="stage", bufs=4) as stage_pool,
            tc.tile_pool(name="stats", bufs=1) as stats_pool,
        ):
            # --- input DMAs (Sync queue) + eye (GpSimd, lands first) ---
            eye = stats_pool.tile([P, P], f16, name="eye")
            nc.gpsimd.dma_start(eye[:], eyed[:])

            lhs_tiles = []
            for a in range(NA):
                t = lhs_pool.tile([P, KCH, NBLK], fp8, name=f"lhs{a}",
                                  tag=f"lhs{a}")
                lhs_tiles.append(t)
                rows = slice(8 * a * P, (8 * a + 1) * P)
                nc.sync.dma_start(t[:], xt_ap[rows, :])

            racc = [
                stats_pool.tile([P, 4, NBLK], f16, name=f"racc{a}",
                                tag=f"racc{a}")
                for a in range(NA)
            ]
            rowmax_sb = stats_pool.tile([P, NA * 4], f32, name="rowmax_sb")
            rowtmp = stats_pool.tile([P, 4], f32, name="rowtmp")
            # scratch for the last block's j-fold tree
            stj = stats_pool.tile([P, 4, NBLK // 2], f16, name="stj")
            stj2 = stats_pool.tile([P, 4, NBLK // 4], f16, name="stj2")

            # --- PE warm-up: tiny f16 matmuls on the eye tile fill the
            # HAM activity window while the first lhs slab streams in ---
            wps = psum_pool.tile([P, 4, NBLK], f32, name="wps", tag="ps")
            for _ in range(N_WARMUP):
                nc.tensor.matmul(wps[:, 0, 0:P], eye[:], eye[:],
                                 start=True, stop=True)

            def row_reduce(dst_ap, src_tile):
                """dst[:, 0:4] = per-chunk row max of src [P, 4, NBLK]."""
                nc.vector.reduce_max(
                    out=dst_ap,
                    in_=src_tile[:],
                    axis=mybir.AxisListType.X,
                    op=mybir.AluOpType.max,
                )

            sb_idx = 0
            for a, w in _windows():
                L = (8 * a + w) % NSB
                wmax = 16 if a < 2 else 15
                if L % 8 == 0:
                    rt = lhs_tiles[L // 8]  # resident (w = 0, 8, 16)
                else:
                    rt = rhs_pool.tile([P, KCH, NBLK], fp8, name="rt",
                                       tag="rt")
                    nc.sync.dma_start(rt[:], xt_ap[L * P:(L + 1) * P, :])

                ps = psum_pool.tile([P, 4, NBLK], f32, name="ps", tag="ps")
                st = stage_pool.tile([P, 4, NBLK], f16, name="st", tag="st")
                if w == 0:
                    # diagonal: chunk r covers block cols 128r..512 only
                    for r in range(4):
                        nw = NBLK - r * P
                        for g in range(ngrp):
                            ks = slice(KSTEP * g, KSTEP * (g + 1))
                            nc.tensor.matmul(
                                ps[:, r, 0:nw],
                                lhs_tiles[a][:, ks, r * P:(r + 1) * P],
                                lhs_tiles[a][:, ks, r * P:NBLK],
                                start=(g == 0),
                                stop=(g == ngrp - 1),
                                perf_mode=mybir.MatmulPerfMode.DoubleRow,
                            )
                    # ragged PSUM->f16 stage copies (ACT)
                    for r in range(4):
                        nw = NBLK - r * P
                        nc.scalar.copy(st[:, r, 0:nw], ps[:, r, 0:nw])
                    # self-sim sits at st[p, r, p]: mask it below any
                    # real similarity before any max consumes it
                    for r in range(4):
                        sl = st[:, r, 0:P]
                        nc.vector.tensor_add(out=sl, in0=sl, in1=eye[:])
                    # init racc: ragged copy + -inf tails
                    nc.vector.tensor_copy(racc[a][:, 0, :], st[:, 0, :])
                    for r in range(1, 4):
                        nw = NBLK - r * P
                        nc.vector.tensor_copy(racc[a][:, r, 0:nw],
                                              st[:, r, 0:nw])
                        nc.vector.memset(racc[a][:, r, nw:NBLK], -60000.0)
                    # ragged column-max over the computed rows, block
                    # cols 128..512 (col offset 128r+j in chunk r maps to
                    # local j after the per-chunk 128-col shift)
                    mgd = stage_pool.tile([P, DIAG_W], f16, name="mgd",
                                          tag="mgd", bufs=2)
                    nc.vector.tensor_max(out=mgd[:],
                                         in0=st[:, 0, P:NBLK],
                                         in1=st[:, 1, 0:DIAG_W])
                    nc.vector.tensor_max(out=mgd[:, P:DIAG_W],
                                         in0=mgd[:, P:DIAG_W],
                                         in1=st[:, 2, 0:NBLK - 2 * P])
                    nc.vector.tensor_max(out=mgd[:, 2 * P:DIAG_W],
                                         in0=mgd[:, 2 * P:DIAG_W],
                                         in1=st[:, 3, 0:P])
                    nc.gpsimd.dma_start(
                        colmax_ap[sb_idx * P:(sb_idx + 1) * P, 0:DIAG_W],
                        mgd[:])
                else:
                    for r in range(4):
                        for g in range(ngrp):
                            ks = slice(KSTEP * g, KSTEP * (g + 1))
                            nc.tensor.matmul(
                                ps[:, r, :],
                                lhs_tiles[a][:, ks, r * P:(r + 1) * P],
                                rt[:, ks, :],
                                start=(g == 0),
                                stop=(g == ngrp - 1),
                                perf_mode=mybir.MatmulPerfMode.DoubleRow,
                            )
                    # ACT is the sole bulk PSUM consumer: one [128, 2048]
                    # copy stages the tile to f16 SBUF so the PE recycles
                    # its two psum buffers quickly.
                    nc.scalar.copy(st[:], ps[:])

                    last = (a == NA - 1 and w == wmax)
                    if not last:
                        # running row-max (f16 2x tensor_tensor)
                        nc.vector.tensor_max(out=racc[a][:],
                                             in0=racc[a][:], in1=st[:])
                    # column-max fold to [128, 512]; host folds partitions
                    stm = stage_pool.tile([P, 2, NBLK], f16, name="stm",
                                          tag="stm", bufs=6)
                    nc.vector.tensor_max(out=stm[:], in0=st[:, 0:2, :],
                                         in1=st[:, 2:4, :])
                    mg = stage_pool.tile([P, NBLK], f16, name="mg",
                                         tag="mg", bufs=8)
                    nc.vector.tensor_max(out=mg[:], in0=stm[:, 0, :],
                                         in1=stm[:, 1, :])
                    nc.gpsimd.dma_start(
                        colmax_ap[sb_idx * P:(sb_idx + 1) * P, :], mg[:])
                    if last:
                        # tail: j-fold this block's st to [P, 4, 128],
                        # small reduce, and merge (racc for this window
                        # was reduced after w = wmax-1) — ~1.6us instead
                        # of racc-update + 2.2us full reduce
                        nc.vector.tensor_max(out=stj[:],
                                             in0=st[:, :, 0:NBLK // 2],
                                             in1=st[:, :, NBLK // 2:NBLK])
                        nc.vector.tensor_max(out=stj2[:],
                                             in0=stj[:, :, 0:NBLK // 4],
                                             in1=stj[:, :, NBLK // 4:NBLK // 2])
                        row_reduce(rowtmp[:], stj2)
                        nc.vector.tensor_max(
                            out=rowmax_sb[:, 4 * a:4 * a + 4],
                            in0=rowmax_sb[:, 4 * a:4 * a + 4],
                            in1=rowtmp[:])
                sb_idx += 1

                # per-window row-max reduction (hidden under the next
                # block's matmuls; for the very last window it runs one
                # block early and the final block merges via rowtmp)
                red_now = (w == wmax - 1) if (a == NA - 1) else (w == wmax)
                if red_now:
                    row_reduce(rowmax_sb[:, 4 * a:4 * a + 4], racc[a])

            nc.sync.dma_start(rowmax_d[:], rowmax_sb[:])

    nc.compile()
    return nc


def prepare_inputs(x):
    """Host prep: normalize (f64), scale+cast fp8, retile, rotate."""
    xd = np.asarray(x, dtype=np.float64)
    norms = np.sqrt(np.einsum("ij,ij->i", xd, xd))
    np.maximum(norms, 1e-12, out=norms)
    xn = xd / norms[:, None]
    xnt = (xn.T * FP8_SCALE).astype(ml_dtypes.float8_e4m3)  # [D, B]
    # retile to [J, p, k, j]: xt_r[J, p, k, j] = xnt[k*128+p, J*512+j]
    xt_r = np.ascontiguousarray(
        xnt.reshape(KCH, P, NSB, NBLK).transpose(2, 1, 0, 3))
    eyef16 = np.ascontiguousarray(
        EYE_VAL * np.eye(P, dtype=np.float32)).astype(np.float16)
    in_maps = []
    for c in range(N_CORES):
        rot = (np.concatenate([xt_r[c:], xt_r[:c]], axis=0) if c
               else xt_r)
        in_maps.append({
            "xt": np.ascontiguousarray(rot).reshape(NSB * P, KCH * NBLK),
            "eyef16": eyef16,
        })
    return in_maps


def postprocess(results):
    """Merge per-core row/column maxima and apply the scalar epilogue."""
    inv = 1.0 / (FP8_SCALE * FP8_SCALE)
    order = _windows()
    maxsim = np.full(B, -np.inf, dtype=np.float64)
    for c in range(N_CORES):
        rm = np.asarray(results[c]["rowmax"], dtype=np.float64)  # [P, 16]
        for a in range(NA):
            for r in range(4):
                g0 = (c + 8 * a) * NBLK + r * P  # global row of partition 0
                sl = slice(g0, g0 + P)
                np.maximum(maxsim[sl], rm[:, 4 * a + r], out=maxsim[sl])
        cmx = np.asarray(results[c]["colmax"]).astype(np.float32)
        cmx = cmx.reshape(N_SLOTS, P, NBLK).max(axis=1).astype(np.float64)
        for s, (a, w) in enumerate(order):
            g0 = ((8 * a + w + c) % NSB) * NBLK
            if w == 0:
                # diag slot: cols 0:384 cover block cols 128..512
                sl = slice(g0 + P, g0 + NBLK)
                np.maximum(maxsim[sl], cmx[s, 0:DIAG_W], out=maxsim[sl])
            else:
                sl = slice(g0, g0 + NBLK)
                np.maximum(maxsim[sl], cmx[s], out=maxsim[sl])
    d2 = 2.0 - 2.0 * (maxsim * inv) + EPS
    loss = -0.5 * np.mean(np.log(d2))
    return np.array(loss, dtype=np.float32)


_NC_CACHE = {}


def _get_nc():
    if "nc" not in _NC_CACHE:
        _NC_CACHE["nc"] = build_nc()
    return _NC_CACHE["nc"]


def kernel(x, **_ignored):
    import time

    nc = _get_nc()
    in_maps = prepare_inputs(x)
    last_exc = None
    for attempt in range(3):
        try:
            res = run_bass_kernel_spmd(nc, in_maps,
                                       core_ids=list(range(N_CORES)))
            return postprocess(res.results)
        except Exception as exc:  # transient NRT/tunnel hiccups
            last_exc = exc
            if attempt < 2:
                time.sleep(30)  # a wedged exec unit takes a while to heal
    raise last_exc


if __name__ == "__main__":
    x = np.random.default_rng(0).standard_normal((B, D), dtype=np.float32)
    print(kernel(x))
